# revision 1
# baseline (speedup 1.0000x reference)
# Trainium2 Bass kernel for nn_ComplementarySystem: two 2-layer conv branches
# (7x7/s2 + 3x3/s2, GAP, fc->2) over x[64,3,224,224], argmax each, spoof/live
# select. Data-parallel over 8 NeuronCores (8 samples each).
#
# Self-contained: only numpy + concourse (bass) imports. No file reads.
import numpy as np

# ---------------- problem constants (hardcoded per spec) ----------------
B = 64
BPC = 8          # samples per core
NCORES = 8
CIN, H, W = 3, 224, 224
C1, C2 = 64, 128
H1, W1 = 112, 112      # conv1 output
H2, W2 = 56, 56        # conv2 output
PL = 116               # padded phase-plane rows/cols (r2 in [-1,114], c2 in [-1,114])
NPOS2 = H2 * W2        # 3136

# conv1 tap decomposition: input row r = 2*r1 + kh - 2, col = 2*ox + kw - 2.
# Virtual tap space = (dx:4) x (dy:4) x (phy:2, phx:2, c:3) = 192 rows;
# kh = 2*dy+2+phy, kw = 2*dx+2+phx; rows with kh>6 or kw>6 are phantoms with
# zero weights. One im2col DMA per dx covers 48 partitions:
# src [[116,4](dy), [13456,12](phy,phx,c), [1,12992](rows*cols flat)].
DYS = [-1, 0, 1, 2]
DXS4 = [-1, 0, 1, 2]


USE_BF16 = True        # compute conv matmuls in bf16 (fp32 accumulate)
DEBUG_DUMP = False     # add dram dumps of h1/G for sample 0 (sim debugging)
REPS = 1               # repeat the whole sample pipeline (wall-clock timing aid)


# ---------------- host-side layout prep (pure data movement) ----------------
def _phase_planes(x):
    """x [b,3,224,224] f32 -> zero-padded stride-2 phase planes
    [b, phy, phx, c, PL, PL] (c contiguous with the plane so im2col DMAs can
    treat (c, rows*cols) as a 2-dim access)."""
    b = x.shape[0]
    p = np.zeros((b, 2, 2, CIN, PL, PL), dtype=np.float32)
    p[:, 0, 0, :, 1:113, 1:113] = x[:, :, 0::2, 0::2]
    p[:, 0, 1, :, 1:113, 1:113] = x[:, :, 0::2, 1::2]
    p[:, 1, 0, :, 1:113, 1:113] = x[:, :, 1::2, 0::2]
    p[:, 1, 1, :, 1:113, 1:113] = x[:, :, 1::2, 1::2]
    return p


def _prep_weights(inp):
    tW1, fW1 = np.asarray(inp["tW1"]), np.asarray(inp["fW1"])   # [64,3,7,7]
    tW2, fW2 = np.asarray(inp["tW2"]), np.asarray(inp["fW2"])   # [128,64,3,3]
    tb1, fb1 = np.asarray(inp["tb1"]), np.asarray(inp["fb1"])
    tb2, fb2 = np.asarray(inp["tb2"]), np.asarray(inp["fb2"])
    tWfc, fWfc = np.asarray(inp["tWfc"]), np.asarray(inp["fWfc"])  # [128,2]
    tbfc, fbfc = np.asarray(inp["tbfc"]), np.asarray(inp["fbfc"])  # [2]

    # virtual 192-tap lhsT: two K=96 blocks (dx0,dx1 | dx2,dx3); row order
    # within a 48-block = (dy, phy, phx, c) matching the im2col DMA dims.
    def w1block(dxs):
        a = np.zeros((96, 2, 128), dtype=np.float32)
        i = 0
        for dx in dxs:
            for dy in DYS:
                for phy in (0, 1):
                    for phx in (0, 1):
                        for c in range(CIN):
                            kh = 2 * dy + 2 + phy
                            kw = 2 * dx + 2 + phx
                            if kh <= 6 and kw <= 6:
                                # parity 0: psum 0-63 = t, 64-127 = f; parity 1
                                # swapped so t lands on psum parts 64-127
                                a[i, 0, 0:64] = tW1[:, c, kh, kw]
                                a[i, 0, 64:128] = fW1[:, c, kh, kw]
                                a[i, 1, 0:64] = fW1[:, c, kh, kw]
                                a[i, 1, 64:128] = tW1[:, c, kh, kw]
                            i += 1
        return a

    w1a = w1block([-1, 0])         # [96,2,128] dx blocks 0,1
    w1b = w1block([1, 2])          # [96,2,128] dx blocks 2,3

    b1p = np.zeros((128, 2), dtype=np.float32)
    b1p[0:64, 0], b1p[64:128, 0] = tb1, fb1    # even rounds
    b1p[0:64, 1], b1p[64:128, 1] = fb1, tb1    # odd rounds

    # conv2 paired (kh=0,1) lhsT per (branch, kw): K rows = h1 partition layout
    # h1_t parts 0-63 = (ch,phy0), 64-127 = (ch,phy1); h1_f is swapped.
    w2p = np.zeros((128, 6, 128), dtype=np.float32)
    for kw in range(3):
        w2p[0:64, kw, :] = tW2[:, :, 0, kw].T        # t: phy0 rows = kh0
        w2p[64:128, kw, :] = tW2[:, :, 1, kw].T      # t: phy1 rows = kh1
        w2p[0:64, 3 + kw, :] = fW2[:, :, 1, kw].T    # f: parts 0-63 = phy1 = kh1
        w2p[64:128, 3 + kw, :] = fW2[:, :, 0, kw].T  # f: parts 64-127 = phy0 = kh0
    # kh2 as full-K=128 matmuls: the other 64 rows are zero-weight phantoms
    # (they read h1 phy1 rows shifted by one, incl. the zeroed border row 56)
    w2k2 = np.zeros((128, 6, 128), dtype=np.float32)
    for kw in range(3):
        w2k2[0:64, kw, :] = tW2[:, :, 2, kw].T       # t kh2 via parts 0-63 (phy0)
        w2k2[64:128, 3 + kw, :] = fW2[:, :, 2, kw].T  # f kh2 via parts 64-127

    b2c = np.stack([tb2, fb2], axis=1).astype(np.float32)       # [128,2]
    wfc = np.stack([tWfc, fWfc], axis=1).astype(np.float32)     # [128,2,2]
    bfc = np.stack([tbfc, fbfc], axis=0)[None].astype(np.float32)  # [1,2,2]
    return dict(w1a=w1a, w1b=w1b, b1p=b1p, w2p=w2p, w2k2=w2k2,
                b2c=b2c, wfc=wfc, bfc=bfc)


# ---------------- device program ----------------
def build_nc():
    import concourse.bass as bass
    import concourse.mybir as mybir
    import concourse.tile as tile
    from concourse import bacc
    from contextlib import ExitStack

    f32 = mybir.dt.float32
    dtc = mybir.dt.bfloat16 if USE_BF16 else f32
    AF = mybir.ActivationFunctionType
    OP = mybir.AluOpType
    AX = mybir.AxisListType

    nc = bacc.Bacc(trn_type="TRN2")
    xp_d = nc.dram_tensor("xp", [BPC, 2, 2, CIN, PL, PL], f32, kind="ExternalInput")
    w1a_d = nc.dram_tensor("w1a", [96, 2, 128], f32, kind="ExternalInput")
    w1b_d = nc.dram_tensor("w1b", [96, 2, 128], f32, kind="ExternalInput")
    b1p_d = nc.dram_tensor("b1p", [128, 2], f32, kind="ExternalInput")
    w2p_d = nc.dram_tensor("w2p", [128, 6, 128], f32, kind="ExternalInput")
    w2k2_d = nc.dram_tensor("w2k2", [128, 6, 128], f32, kind="ExternalInput")
    b2c_d = nc.dram_tensor("b2c", [128, 2], f32, kind="ExternalInput")
    wfc_d = nc.dram_tensor("wfc", [128, 2, 2], f32, kind="ExternalInput")
    bfc_d = nc.dram_tensor("bfc", [1, 2, 2], f32, kind="ExternalInput")
    out_d = nc.dram_tensor("out", [BPC, 2], f32, kind="ExternalOutput")
    marg_d = nc.dram_tensor("marg", [2, BPC], f32, kind="ExternalOutput")
    if DEBUG_DUMP:
        dbg_h1t_d = nc.dram_tensor("dbg_h1t", [128, 57, 113], f32,
                                   kind="ExternalOutput")
        dbg_h1f_d = nc.dram_tensor("dbg_h1f", [128, 57, 113], f32,
                                   kind="ExternalOutput")
        dbg_G_d = nc.dram_tensor("dbg_G", [128, 2, BPC], f32,
                                 kind="ExternalOutput")
        dbg_h2_d = nc.dram_tensor("dbg_h2", [128, 7, 56], f32,
                                  kind="ExternalOutput")

    # dram element strides of xp [s, phy, phx, c, PL, PL]
    XS_C = PL * PL
    XS_PHX = CIN * XS_C
    XS_PHY = 2 * XS_PHX
    XS_S = 2 * XS_PHY

    with ExitStack() as ctx:
        tc = ctx.enter_context(tile.TileContext(nc))
        wp = ctx.enter_context(tc.tile_pool(name="weights", bufs=1))
        imp = ctx.enter_context(tc.tile_pool(name="im", bufs=2 if USE_BF16 else 1))
        h1p = ctx.enter_context(tc.tile_pool(name="h1", bufs=2 if USE_BF16 else 1))
        scp = ctx.enter_context(tc.tile_pool(name="scratch", bufs=3))
        gp = ctx.enter_context(tc.tile_pool(name="gap", bufs=2))
        pp1 = ctx.enter_context(tc.tile_pool(name="ps1", bufs=3, space="PSUM"))
        pp2 = ctx.enter_context(tc.tile_pool(name="ps2", bufs=2, space="PSUM"))
        ppf = ctx.enter_context(tc.tile_pool(name="psf", bufs=1, space="PSUM"))

        # ---- load + (optionally cast) weights ----
        def load_w(name, dram, shape, cast):
            if cast and USE_BF16:
                t0 = wp.tile(shape, dtc, tag=f"w_{name}")
                nc.gpsimd.dma_start(out=t0, in_=dram.ap())  # SWDGE cast f32->bf16
            else:
                t0 = wp.tile(shape, f32, tag=f"w_{name}")
                nc.sync.dma_start(t0, dram.ap())
            return t0

        # bf16 copy of the input planes (one-shot dram->dram cast via SWDGE)
        if USE_BF16:
            xpb_d = nc.dram_tensor("xpb", [BPC, 2, 2, CIN, PL, PL], dtc,
                                   kind="Internal")
            nc.gpsimd.dma_start(out=xpb_d.ap(), in_=xp_d.ap())
            im_src = xpb_d
        else:
            im_src = xp_d

        w1a = load_w("w1a", w1a_d, [96, 2, 128], True)
        w1b = load_w("w1b", w1b_d, [96, 2, 128], True)
        w2p = load_w("w2p", w2p_d, [128, 6, 128], True)
        w2k2 = load_w("w2k2", w2k2_d, [128, 6, 128], True)
        b1p = load_w("b1p", b1p_d, [128, 2], False)
        b2c = load_w("b2c", b2c_d, [128, 2], False)
        wfc = load_w("wfc", wfc_d, [128, 2, 2], False)
        bfc = load_w("bfc", bfc_d, [1, 2, 2], False)

        G = wp.tile([128, 2, BPC], f32)   # GAP sums per (ch, branch, sample)

        for s in [s for _ in range(REPS) for s in range(BPC)]:
            h1t = h1p.tile([128, 57, 113], dtc, tag="h1t")
            h1f = h1p.tile([128, 57, 113], dtc, tag="h1f")
            # zero borders: phy0 row 56 (kh=2 reach at oy=55) + col 112 (kw reach)
            nc.vector.memset(h1t[0:64, 56, :], 0.0)
            nc.vector.memset(h1f[64:128, 56, :], 0.0)
            nc.vector.memset(h1t[64:128, 56, :], 0.0)
            nc.vector.memset(h1f[0:64, 56, :], 0.0)
            nc.vector.memset(h1t[:, :, 112:113], 0.0)
            nc.vector.memset(h1f[:, :, 112:113], 0.0)

            # ---- build full-sample im2col in DRAM (fat reshuffle; dram->dram
            # DMAs have no partition-thinness penalty), then two fat
            # 84/63-partition loads into SBUF. ----
            # im2col tile row per virtual tap = 112 rows x 116 cols (full
            # plane-width rows; (rows, cols) merge into one flat 12992-elem
            # run, last 1+dx cols of each row are wrap junk never read). ONE
            # dram->sbuf DMA per dx covers 48 partitions (dy x all 12 planes).
            FLAT = 112 * PL
            im0 = imp.tile([96, 112, PL], dtc, tag="im0")
            im1 = imp.tile([96, 112, PL], dtc, tag="im1")
            for di, dx in enumerate(DXS4):
                base = s * XS_S + (1 + dx)
                src = bass.AP(
                    tensor=im_src, offset=base,
                    ap=[[PL, 4], [XS_C, 12], [1, FLAT]])
                tile_t = im0 if di < 2 else im1
                pb = 48 * (di % 2)
                eng = nc.sync if di % 2 == 0 else nc.gpsimd
                eng.dma_start(out=tile_t[pb:pb + 48], in_=src)

            for par in (0, 1):
                # ---- conv1 matmuls: 14 chunks of 4 same-parity rows ----
                for j in range(14):
                    ps = pp1.tile([128, 4, 112], f32, tag="c1")
                    nc.tensor.matmul(ps, w1a[:, par, :],
                                     im0[:, par + 8 * j:par + 8 * j + 7:2, 0:112],
                                     start=True, stop=False)
                    nc.tensor.matmul(ps, w1b[:, par, :],
                                     im1[:, par + 8 * j:par + 8 * j + 7:2, 0:112],
                                     start=False, stop=True)
                    jq = 4 * j
                    # per-parity psum->h1 mapping (even: t low / f high; odd
                    # rounds swapped via lhsT columns). Alternate which engine
                    # (ACT vs DVE) takes which half per chunk to balance load.
                    if par == 0:
                        halves = [
                            (h1t[0:64, jq:jq + 4, 0:112], ps[0:64],
                             b1p[0:64, 0:1]),
                            (h1f[64:128, jq:jq + 4, 0:112], ps[64:128],
                             b1p[64:128, 0:1]),
                        ]
                    else:
                        halves = [
                            (h1t[64:128, jq:jq + 4, 0:112], ps[64:128],
                             b1p[64:128, 1:2]),
                            (h1f[0:64, jq:jq + 4, 0:112], ps[0:64],
                             b1p[0:64, 1:2]),
                        ]
                    a, d = (halves[0], halves[1]) if j % 2 == 0 else \
                           (halves[1], halves[0])
                    if j % 6 == 5:   # DVE takes both (ACT also carries conv2)
                        nc.vector.tensor_scalar(out=a[0], in0=a[1], scalar1=a[2],
                                                scalar2=0.0, op0=OP.add,
                                                op1=OP.max)
                    else:
                        nc.scalar.activation(out=a[0], in_=a[1], func=AF.Relu,
                                             bias=a[2])
                    nc.vector.tensor_scalar(out=d[0], in0=d[1], scalar1=d[2],
                                            scalar2=0.0, op0=OP.add, op1=OP.max)

            # ---- conv2: 8 chunks of 7 oy-rows (392 cols), branches interleaved
            # so the two K=64 kh2 matmuls (rows 0-63 vs 64-127) run
            # concurrently in the PE via disjoint row groups.
            gcols = gp.tile([128, 2, 8], f32, tag="gc")
            for c8 in range(8):
                oy0 = 7 * c8
                pst = pp2.tile([128, 7, 56], f32, tag="c2t")
                psf = pp2.tile([128, 7, 56], f32, tag="c2f")
                for br, ps2, h1b in ((0, pst, h1t), (1, psf, h1f)):
                    for kw in range(3):
                        nc.tensor.matmul(
                            ps2, w2p[:, 3 * br + kw, :],
                            h1b[:, oy0:oy0 + 7, kw:kw + 111:2],
                            start=(kw == 0), stop=False)
                for kw in range(3):
                    nc.tensor.matmul(
                        pst, w2k2[:, kw, :],
                        h1t[:, oy0 + 1:oy0 + 8, kw:kw + 111:2],
                        start=False, stop=(kw == 2))
                    nc.tensor.matmul(
                        psf, w2k2[:, 3 + kw, :],
                        h1f[:, oy0 + 1:oy0 + 8, kw:kw + 111:2],
                        start=False, stop=(kw == 2))
                for br, ps2 in ((0, pst), (1, psf)):
                    scr = scp.tile([128, 7, 56], f32, tag="h2scr")
                    nc.scalar.activation(
                        out=scr, in_=ps2, func=AF.Relu,
                        bias=b2c[:, br:br + 1], accum_out=gcols[:, br, c8:c8 + 1])
                    if DEBUG_DUMP and s == 0 and br == 0 and c8 == 0:
                        nc.sync.dma_start(out=dbg_h2_d.ap(), in_=scr)
            nc.vector.reduce_sum(out=G[:, 0, s:s + 1], in_=gcols[:, 0, :],
                                 axis=AX.X)
            nc.vector.reduce_sum(out=G[:, 1, s:s + 1], in_=gcols[:, 1, :],
                                 axis=AX.X)
            if DEBUG_DUMP and s == 0:
                if USE_BF16:
                    h1tc = scp.tile([128, 57, 113], f32, tag="h1dump")
                    nc.vector.tensor_copy(h1tc, h1t)
                    nc.sync.dma_start(out=dbg_h1t_d.ap(), in_=h1tc)
                    h1fc = scp.tile([128, 57, 113], f32, tag="h1dump")
                    nc.vector.tensor_copy(h1fc, h1f)
                    nc.sync.dma_start(out=dbg_h1f_d.ap(), in_=h1fc)
                else:
                    nc.sync.dma_start(out=dbg_h1t_d.ap(), in_=h1t)
                    nc.sync.dma_start(out=dbg_h1f_d.ap(), in_=h1f)

        if DEBUG_DUMP:
            nc.sync.dma_start(out=dbg_G_d.ap(), in_=G)
        # ---- fc + decision tail ----
        wd = scp.tile([128, 2], f32, tag="wd")
        nc.vector.tensor_tensor(out=wd, in0=wfc[:, :, 1], in1=wfc[:, :, 0],
                                op=OP.subtract)
        nc.scalar.mul(out=wd, in_=wd, mul=1.0 / NPOS2)
        bd = scp.tile([1, 2], f32, tag="bd")
        nc.vector.tensor_tensor(out=bd, in0=bfc[0:1, :, 1], in1=bfc[0:1, :, 0],
                                op=OP.subtract)
        psfc = ppf.tile([1, 2, 8], f32, tag="fc")
        nc.tensor.matmul(psfc[0:1, 0, :], wd[:, 0:1], G[:, 0, :],
                         start=True, stop=False, skip_group_check=True)
        nc.tensor.matmul(psfc[0:1, 1, :], wd[:, 1:2], G[:, 1, :],
                         start=False, stop=True, skip_group_check=True)
        pst, psf = psfc[0:1, 0, :], psfc[0:1, 1, :]
        d = scp.tile([1, 2, 8], f32, tag="d")
        nc.scalar.activation(out=d[0:1, 0, :], in_=pst, func=AF.Identity,
                             bias=bd[0:1, 0:1])
        nc.scalar.activation(out=d[0:1, 1, :], in_=psf, func=AF.Identity,
                             bias=bd[0:1, 1:2])
        nc.sync.dma_start(out=marg_d.ap(), in_=d[0:1].rearrange("p a b -> p (a b)"))
        m = scp.tile([1, 8], f32, tag="m")
        nc.vector.tensor_tensor(out=m, in0=d[0:1, 0, :], in1=d[0:1, 1, :],
                                op=OP.max)
        g = scp.tile([1, 8], f32, tag="g")
        nc.vector.tensor_scalar(out=g, in0=m, scalar1=0.0, scalar2=None,
                                op0=OP.is_gt)
        oi = scp.tile([1, 8, 2], f32, tag="oi")
        nc.vector.tensor_scalar(out=oi[0:1, :, 0], in0=g, scalar1=-20.0,
                                scalar2=10.0, op0=OP.mult, op1=OP.add)
        nc.vector.tensor_scalar(out=oi[0:1, :, 1], in0=g, scalar1=20.0,
                                scalar2=-10.0, op0=OP.mult, op1=OP.add)
        nc.sync.dma_start(out=out_d.ap(), in_=oi[0:1].rearrange("p a b -> p (a b)"))

    nc.compile()
    return nc


_NC_CACHE = {}


def get_nc():
    key = (USE_BF16, REPS, DEBUG_DUMP)
    if key not in _NC_CACHE:
        _NC_CACHE[key] = build_nc()
    return _NC_CACHE[key]


def make_in_maps(inputs):
    x = np.asarray(inputs["x"], dtype=np.float32)
    planes = _phase_planes(x)                       # [64,3,2,2,PL,PL]
    wts = _prep_weights(inputs)
    in_maps = []
    for k in range(NCORES):
        m = dict(wts)
        m["xp"] = np.ascontiguousarray(planes[k * BPC:(k + 1) * BPC])
        in_maps.append(m)
    return in_maps


def kernel(**inputs):
    from concourse.bass_utils import run_bass_kernel_spmd
    nc = get_nc()
    in_maps = make_in_maps(inputs)
    res = run_bass_kernel_spmd(nc, in_maps, core_ids=list(range(NCORES)))
    out = np.concatenate([r["out"] for r in res.results], axis=0)
    return out.astype(np.float32)

